# revision 2
# baseline (speedup 1.0000x reference)
"""BinLinear TRN2 kernel: out = x @ sign(weight).T + sign(bias).

Full shapes: x [8192, 4096] f32, weight [4096, 4096] f32, bias [4096] f32
-> out [8192, 4096] f32.

Sharding (8 NeuronCores): 2D grid, 4-way over tokens x 2-way over output
features. Each core computes out_c = x_c @ sign(w_c).T + sign(b_c) with
x_c [2048, 4096], w_c [2048, 4096], b_c [2048] -> out_c [2048, 2048].
The host only slices inputs and stitches the 4x2 output grid back together.

Per-core device program (everything on device, fp16 single-pass matmul):
  - x is cast fp32->fp16 by SWDGE DMA (DRAM->DRAM), then loaded transposed
    via XBAR dma-transpose into SBUF tiles xT[i-part, kt, tok].
  - w tiles are loaded [o, i], binarized on DVE in one tensor_scalar op
    (w > 0) - 0.5 = +-0.5 (exact when w has no zeros; a 3-op exact-sign
    variant handles zeros), then XBAR-transposed SBUF->SBUF into resident
    wT[i-part, kt, o] tiles.
  - PE: for each [128-token, 512-feature] PSUM tile, one K=1 matmul
    accumulates the bias row (ones^T @ (+-0.5 bias sign)), then 32 K=128
    fp16 matmuls accumulate x @ w_sign.  Copy-back scales by 2 on DVE
    (recovering +-1 weights/bias) straight into fp32 output tiles.
"""

import sys

if "/opt/trn_rl_repo" not in sys.path:
    sys.path.insert(0, "/opt/trn_rl_repo")

from contextlib import ExitStack

import numpy as np

import concourse.bass as bass
import concourse.mybir as mybir
import concourse.tile as tile
from concourse import bacc
from concourse.bass_utils import run_bass_kernel_spmd

N_TOK, D_IN, D_OUT = 8192, 4096, 4096
TOK_WAYS, OUT_WAYS = 4, 2
N_CORES = TOK_WAYS * OUT_WAYS
TOK_SH = N_TOK // TOK_WAYS    # 2048 tokens per core
OUT_SH = D_OUT // OUT_WAYS    # 2048 out features per core

P = 128
KT = D_IN // P                # 32 contraction subtiles
NFREE = 512                   # PSUM free dim per matmul
TB = TOK_SH // P              # 16 token tiles
NB = OUT_SH // NFREE          # 4 feature blocks
RPB = NFREE // P              # 4 weight row-tiles per feature block

F16 = mybir.dt.float16
F32 = mybir.dt.float32


def _build(exact_sign: bool):
    """Build the per-core SPMD program."""
    nc = bacc.Bacc("TRN2", target_bir_lowering=False, debug=False,
                   num_devices=N_CORES)
    x = nc.dram_tensor("x", [TOK_SH, D_IN], F32, kind="ExternalInput")
    w = nc.dram_tensor("w", [OUT_SH, D_IN], F32, kind="ExternalInput")
    b = nc.dram_tensor("b", [1, OUT_SH], F32, kind="ExternalInput")
    out = nc.dram_tensor("out", [TOK_SH, OUT_SH], F32, kind="ExternalOutput")

    with ExitStack() as ctx:
        tc = ctx.enter_context(tile.TileContext(nc))
        dram = ctx.enter_context(tc.tile_pool(name="dram", bufs=TB, space="DRAM"))
        wTp = ctx.enter_context(tc.tile_pool(name="wT", bufs=NB))
        xTp = ctx.enter_context(tc.tile_pool(name="xT", bufs=2))
        wf32p = ctx.enter_context(tc.tile_pool(name="wf32", bufs=1))
        f16tmp = ctx.enter_context(tc.tile_pool(name="f16tmp", bufs=3 if exact_sign else 2))
        osbp = ctx.enter_context(tc.tile_pool(name="osb", bufs=2))
        constp = ctx.enter_context(tc.tile_pool(name="const", bufs=1))
        mmps = ctx.enter_context(tc.tile_pool(name="mmps", bufs=4, space="PSUM"))

        def sign_tile(dst_f16, src_f32, tmp_pool, tag):
            """dst = 0.5*sign(src) (exact variant) or (src>0)-0.5 (fast)."""
            if exact_sign:
                t1 = tmp_pool.tile(list(dst_f16.shape), F16, tag=tag)
                t2 = tmp_pool.tile(list(dst_f16.shape), F16, tag=tag)
                nc.vector.tensor_scalar(t1[:], src_f32, 0.0, None, mybir.AluOpType.is_gt)
                nc.vector.tensor_scalar(t2[:], src_f32, 0.0, None, mybir.AluOpType.is_lt)
                # (t1 - t2) * 0.5 -> {-0.5, 0, +0.5}; tensor_tensor then scale
                nc.vector.tensor_tensor(t1[:], t1[:], t2[:], mybir.AluOpType.subtract)
                nc.vector.tensor_scalar(dst_f16, t1[:], 0.5, None, mybir.AluOpType.mult)
            else:
                nc.vector.tensor_scalar(
                    dst_f16, src_f32, 0.0, 0.5,
                    mybir.AluOpType.is_gt, mybir.AluOpType.subtract,
                )

        # ---- constants: ones row for the bias matmul
        ones = constp.tile([1, P], F16)
        nc.gpsimd.memset(ones[:], 1.0)

        # ---- bias: brow = 0.5*sign(b) as fp16 [1, OUT_SH]
        bf32 = constp.tile([1, OUT_SH], F32)
        nc.scalar.dma_start(bf32[:], b[:])
        brow = constp.tile([1, OUT_SH], F16)
        sign_tile(brow[:], bf32[:], constp, "btmp")

        # ---- x: SWDGE cast fp32 -> fp16 chunks in DRAM
        x16 = []
        for t in range(TB):
            ch = dram.tile([P, D_IN], F16, tag="x16")
            nc.gpsimd.dma_start(ch[:], x[t * P : (t + 1) * P, :])
            x16.append(ch)

        # ---- W: load [128, D_IN] row tiles, sign on DVE, XBAR SBUF->SBUF
        #      transpose into resident wT[nb][i-part, kt, o 512]
        wT = [wTp.tile([P, KT, NFREE], F16, tag="wT", name=f"wT{i}")
              for i in range(NB)]
        for r in range(OUT_SH // P):
            wf32 = wf32p.tile([P, D_IN], F32, tag="wf32")
            nc.scalar.dma_start(wf32[:], w[r * P : (r + 1) * P, :])
            wsgn = f16tmp.tile([P, D_IN], F16, tag="wsgn")
            sign_tile(wsgn[:], wf32[:], f16tmp, "wsgn")
            nb, rr = divmod(r, RPB)
            nc.sync.dma_start_transpose(wT[nb][:, :, rr * P : (rr + 1) * P], wsgn[:])

        # ---- main loop: per 128-token tile, 4 psum blocks of 512 features
        for t in range(TB):
            xT = xTp.tile([P, KT, P], F16, tag="xT")
            nc.sync.dma_start_transpose(xT[:], x16[t][:])
            for nb in range(NB):
                psum = mmps.tile([P, NFREE], F32, tag="mm")
                # bias row: psum = ones^T @ brow-slice  (K=1 matmul)
                nc.tensor.matmul(
                    psum[:], ones[:], brow[:, nb * NFREE : (nb + 1) * NFREE],
                    start=True, stop=False,
                )
                for kt in range(KT):
                    nc.tensor.matmul(
                        psum[:], xT[:, kt, :], wT[nb][:, kt, :],
                        start=False, stop=(kt == KT - 1),
                    )
                osb = osbp.tile([P, NFREE], F32, tag="osb")
                # undo the 0.5 scaling of weights+bias
                nc.vector.tensor_scalar(osb[:], psum[:], 2.0, None, mybir.AluOpType.mult)
                nc.scalar.dma_start(
                    out[t * P : (t + 1) * P, nb * NFREE : (nb + 1) * NFREE], osb[:]
                )

    nc.finalize()
    return nc


_cache = {}


def _get_nc(exact_sign: bool):
    if exact_sign not in _cache:
        _cache[exact_sign] = _build(exact_sign)
    return _cache[exact_sign]


def kernel(x: np.ndarray, weight: np.ndarray, bias: np.ndarray) -> np.ndarray:
    x = np.ascontiguousarray(np.asarray(x, dtype=np.float32))
    weight = np.ascontiguousarray(np.asarray(weight, dtype=np.float32))
    bias = np.ascontiguousarray(np.asarray(bias, dtype=np.float32))
    assert x.shape == (N_TOK, D_IN) and weight.shape == (D_OUT, D_IN)

    # (w > 0) - 0.5 equals 0.5*sign(w) only when no exact zeros exist;
    # fall back to the exact 3-op sign variant otherwise.
    exact_sign = bool((weight == 0.0).any() or (bias == 0.0).any())
    nc = _get_nc(exact_sign)

    in_maps = []
    for tg in range(TOK_WAYS):
        for og in range(OUT_WAYS):
            in_maps.append({
                "x": np.ascontiguousarray(x[tg * TOK_SH : (tg + 1) * TOK_SH, :]),
                "w": np.ascontiguousarray(weight[og * OUT_SH : (og + 1) * OUT_SH, :]),
                "b": np.ascontiguousarray(bias[og * OUT_SH : (og + 1) * OUT_SH].reshape(1, OUT_SH)),
            })

    res = run_bass_kernel_spmd(nc, in_maps, list(range(N_CORES)))

    out = np.empty((N_TOK, D_OUT), dtype=np.float32)
    c = 0
    for tg in range(TOK_WAYS):
        for og in range(OUT_WAYS):
            out[tg * TOK_SH : (tg + 1) * TOK_SH, og * OUT_SH : (og + 1) * OUT_SH] = \
                res.results[c]["out"]
            c += 1
    return out


# revision 5
# speedup vs baseline: 1.0539x; 1.0539x over previous
"""BinLinear TRN2 kernel: out = x @ sign(weight).T + sign(bias).

Full shapes: x [8192, 4096] f32, weight [4096, 4096] f32, bias [4096] f32
-> out [8192, 4096] f32.

Sharding (8 NeuronCores): 2D grid, 4-way over tokens x 2-way over output
features. Each core computes out_c = x_c @ sign(w_c).T + sign(b_c) with
x_c [2048, 4096], w_c [2048, 4096], b_c [2048] -> out_c [2048, 2048].
The host only slices inputs and stitches the 4x2 output grid back together.

Per-core device program (fp16 single-pass matmul, everything on device):
  - x is cast fp32->fp16 by SWDGE DMA (DRAM->DRAM, paced behind the
    weight-prep pipeline so it doesn't starve it), then loaded transposed
    via XBAR dma-transpose into SBUF tiles xT[i-part, kt, tok].
  - w row-tiles are loaded [o, i] on the ACT HWDGE ring, binarized on DVE
    ((w > 0) - 0.5 = 0.5*sign, exact when w has no zeros; 3-op exact-sign
    variant otherwise), then XBAR-transposed SBUF->SBUF on the SP ring
    into resident wT[i-part, kt, o] tiles.
  - PE: per [128-token, 512-feature] PSUM tile: one K=1 matmul seeds the
    bias row (ones^T @ 0.5*sign(b)), then 32 K=128 fp16 matmuls accumulate
    x @ w_sign. Copy-back scales by 2 on DVE into fp32 output tiles.
  - Schedule: the first 4 token-tiles run feature-block-major so matmuls
    start as soon as the first quarter of wT is ready; the remaining 12
    token-tiles run token-major with one xT load per tile.
"""

import sys

if "/opt/trn_rl_repo" not in sys.path:
    sys.path.insert(0, "/opt/trn_rl_repo")

from contextlib import ExitStack

import numpy as np

import concourse.bass as bass
import concourse.mybir as mybir
import concourse.tile as tile
from concourse import bacc
from concourse.bass_utils import run_bass_kernel_spmd
from concourse.tile_rust import add_dep_helper

N_TOK, D_IN, D_OUT = 8192, 4096, 4096
TOK_WAYS, OUT_WAYS = 4, 2
N_CORES = TOK_WAYS * OUT_WAYS
TOK_SH = N_TOK // TOK_WAYS    # 2048 tokens per core
OUT_SH = D_OUT // OUT_WAYS    # 2048 out features per core

P = 128
KT = D_IN // P                # 32 contraction subtiles
NFREE = 512                   # PSUM free dim per matmul
TB = TOK_SH // P              # 16 token tiles
NB = OUT_SH // NFREE          # 4 feature blocks
RPB = NFREE // P              # 4 weight row-tiles per feature block
TB_P1 = 4                     # token tiles handled in the nb-major phase

F16 = mybir.dt.float16
F32 = mybir.dt.float32


def _build(exact_sign: bool):
    """Build the per-core SPMD program."""
    nc = bacc.Bacc("TRN2", target_bir_lowering=False, debug=False,
                   num_devices=N_CORES)
    x = nc.dram_tensor("x", [TOK_SH, D_IN], F32, kind="ExternalInput")
    w = nc.dram_tensor("w", [OUT_SH, D_IN], F32, kind="ExternalInput")
    b = nc.dram_tensor("b", [1, OUT_SH], F32, kind="ExternalInput")
    out = nc.dram_tensor("out", [TOK_SH, OUT_SH], F32, kind="ExternalOutput")

    with ExitStack() as ctx:
        tc = ctx.enter_context(tile.TileContext(nc))
        dram = ctx.enter_context(tc.tile_pool(name="dram", bufs=TB, space="DRAM"))
        wTp = ctx.enter_context(tc.tile_pool(name="wTp", bufs=NB))
        xTp = ctx.enter_context(tc.tile_pool(name="xTp", bufs=2))
        wf32p = ctx.enter_context(
            tc.tile_pool(name="wf32p", bufs=1 if exact_sign else 2))
        f16tmp = ctx.enter_context(
            tc.tile_pool(name="f16tmp", bufs=3 if exact_sign else 2))
        osbp = ctx.enter_context(tc.tile_pool(name="osbp", bufs=2))
        constp = ctx.enter_context(tc.tile_pool(name="constp", bufs=1))
        mmps = ctx.enter_context(tc.tile_pool(name="mmps", bufs=6, space="PSUM"))

        def sign_half_tile(dst_f16, src_f32, tmp_pool, tag):
            """dst = 0.5*sign(src) (exact) or (src>0)-0.5 (fast, zero-free)."""
            if exact_sign:
                t1 = tmp_pool.tile(list(dst_f16.shape), F16, tag=tag, name=f"{tag}_t1")
                t2 = tmp_pool.tile(list(dst_f16.shape), F16, tag=tag, name=f"{tag}_t2")
                nc.vector.tensor_scalar(t1[:], src_f32, 0.0, None, mybir.AluOpType.is_gt)
                nc.vector.tensor_scalar(t2[:], src_f32, 0.0, None, mybir.AluOpType.is_lt)
                nc.vector.tensor_tensor(t1[:], t1[:], t2[:], mybir.AluOpType.subtract)
                nc.vector.tensor_scalar(dst_f16, t1[:], 0.5, None, mybir.AluOpType.mult)
            else:
                nc.vector.tensor_scalar(
                    dst_f16, src_f32, 0.0, 0.5,
                    mybir.AluOpType.is_gt, mybir.AluOpType.subtract,
                )

        # ---- constants
        ones = constp.tile([1, P], F16)
        nc.gpsimd.memset(ones[:], 1.0)

        # ---- bias row: brow = 0.5*sign(b) as fp16 [1, OUT_SH]
        bf32 = wf32p.tile([1, OUT_SH], F32, tag="wf32", name="bf32")
        nc.scalar.dma_start(bf32[:], b[:])
        brow = constp.tile([1, OUT_SH], F16)
        sign_half_tile(brow[:], bf32[:], constp, "btmp")

        # ---- W prep: row-tiles [128, D_IN]; loads on ACT ring, sign on DVE,
        #      SBUF->SBUF XBAR transpose on SP ring into resident wT tiles.
        wT = [wTp.tile([P, KT, NFREE], F16, tag="wT", name=f"wT{i}")
              for i in range(NB)]
        for r in range(OUT_SH // P):
            wf32 = wf32p.tile([P, D_IN], F32, tag="wf32", name="wf32")
            nc.scalar.dma_start(wf32[:], w[r * P : (r + 1) * P, :])
            wsgn = f16tmp.tile([P, D_IN], F16, tag="wsgn", name="wsgn")
            sign_half_tile(wsgn[:], wf32[:], f16tmp, "wsgn")
            nb, rr = divmod(r, RPB)
            nc.sync.dma_start_transpose(wT[nb][:, :, rr * P : (rr + 1) * P], wsgn[:])

        # ---- x: SWDGE cast fp32 -> fp16 chunks in DRAM.
        # Chunks 0-1 start immediately; later ones are paced behind xT XBAR
        # loads so the cast flood doesn't starve W prep on the SDMA engines.
        x16 = [None] * TB
        cast_issued = [False] * TB

        def issue_cast(t, dep=None):
            if cast_issued[t]:
                return
            cast_issued[t] = True
            ch = dram.tile([P, D_IN], F16, tag="x16", name=f"x16_{t}")
            inst = nc.gpsimd.dma_start(ch[:], x[t * P : (t + 1) * P, :])
            if dep is not None:
                add_dep_helper(inst.ins, dep.ins, sync=True,
                               reason="pace x cast behind xT consumption")
            x16[t] = ch

        issue_cast(0)
        issue_cast(1)

        def mm_block(t, nb, xT):
            """One [128-token, 512-feature] output block."""
            psum = mmps.tile([P, NFREE], F32, tag="mm", name="psum")
            nc.tensor.matmul(
                psum[:], ones[:], brow[:, nb * NFREE : (nb + 1) * NFREE],
                start=True, stop=False,
            )
            for kt in range(KT):
                nc.tensor.matmul(
                    psum[:], xT[:, kt, :], wT[nb][:, kt, :],
                    start=False, stop=(kt == KT - 1),
                )
            osb = osbp.tile([P, NFREE], F32, tag="osb", name="osb")
            nc.vector.tensor_scalar(osb[:], psum[:], 2.0, None, mybir.AluOpType.mult)
            nc.scalar.dma_start(
                out[t * P : (t + 1) * P, nb * NFREE : (nb + 1) * NFREE], osb[:]
            )

        first_load_done = [False] * TB

        def load_xT(t):
            xT = xTp.tile([P, KT, P], F16, tag="xT", name=f"xT_{t}")
            inst = nc.sync.dma_start_transpose(xT[:], x16[t][:])
            if not first_load_done[t]:
                first_load_done[t] = True
                if t + 2 < TB:
                    issue_cast(t + 2, dep=inst)
            return xT

        # Phase 1: first TB_P1 token tiles, feature-block-major, so matmuls
        # start as soon as wT[0] (W rows 0-3) is ready.
        for nb in range(NB):
            for t in range(TB_P1):
                mm_block(t, nb, load_xT(t))

        # Phase 2: remaining token tiles, token-major (one xT load each).
        for t in range(TB_P1, TB):
            xT = load_xT(t)
            for nb in range(NB):
                mm_block(t, nb, xT)

    nc.finalize()
    return nc


_cache = {}


def _get_nc(exact_sign: bool):
    if exact_sign not in _cache:
        _cache[exact_sign] = _build(exact_sign)
    return _cache[exact_sign]


def kernel(x: np.ndarray, weight: np.ndarray, bias: np.ndarray) -> np.ndarray:
    x = np.ascontiguousarray(np.asarray(x, dtype=np.float32))
    weight = np.ascontiguousarray(np.asarray(weight, dtype=np.float32))
    bias = np.ascontiguousarray(np.asarray(bias, dtype=np.float32))
    assert x.shape == (N_TOK, D_IN) and weight.shape == (D_OUT, D_IN)

    # (w > 0) - 0.5 equals 0.5*sign(w) only when no exact zeros exist;
    # fall back to the exact 3-op sign variant otherwise.
    exact_sign = bool((weight == 0.0).any() or (bias == 0.0).any())
    nc = _get_nc(exact_sign)

    in_maps = []
    for tg in range(TOK_WAYS):
        for og in range(OUT_WAYS):
            in_maps.append({
                "x": np.ascontiguousarray(x[tg * TOK_SH : (tg + 1) * TOK_SH, :]),
                "w": np.ascontiguousarray(weight[og * OUT_SH : (og + 1) * OUT_SH, :]),
                "b": np.ascontiguousarray(bias[og * OUT_SH : (og + 1) * OUT_SH].reshape(1, OUT_SH)),
            })

    res = run_bass_kernel_spmd(nc, in_maps, list(range(N_CORES)))

    out = np.empty((N_TOK, D_OUT), dtype=np.float32)
    c = 0
    for tg in range(TOK_WAYS):
        for og in range(OUT_WAYS):
            out[tg * TOK_SH : (tg + 1) * TOK_SH, og * OUT_SH : (og + 1) * OUT_SH] = \
                res.results[c]["out"]
            c += 1
    return out


# revision 9
# speedup vs baseline: 1.0721x; 1.0173x over previous
"""BinLinear TRN2 kernel: out = x @ sign(weight).T + sign(bias).

Full shapes: x [8192, 4096] f32, weight [4096, 4096] f32, bias [4096] f32
-> out [8192, 4096] f32.

Sharding (8 NeuronCores): 2D grid, 4-way over tokens x 2-way over output
features. Each core computes out_c = x_c @ sign(w_c).T + sign(b_c) with
x_c [2048, 4096], w_c [2048, 4096], b_c [2048] -> out_c [2048, 2048].
The host only slices inputs and stitches the 4x2 output grid back together.

Per-core device program (fp16 single-pass matmul, everything on device):
  - x is cast fp32->fp16 by SWDGE DMA (DRAM->DRAM, paced behind the
    weight-prep pipeline so it doesn't starve it), then loaded transposed
    via XBAR dma-transpose into SBUF tiles xT[i-part, kt, tok].
  - w row-tiles are loaded [o, i] on the ACT HWDGE ring, binarized on DVE
    ((w > 0) - 0.5 = 0.5*sign, exact when w has no zeros; 3-op exact-sign
    variant otherwise), then XBAR-transposed SBUF->SBUF on the SP ring
    into resident wT[i-part, kt, o] tiles.
  - PE: per [128-token, 512-feature] PSUM tile: one K=1 matmul seeds the
    bias row (ones^T @ 0.5*sign(b)), then 32 K=128 fp16 matmuls accumulate
    x @ w_sign. Copy-back scales by 2 on DVE into fp32 output tiles.
  - Schedule: the first 4 token-tiles run feature-block-major so matmuls
    start as soon as the first quarter of wT is ready; the remaining 12
    token-tiles run token-major with one xT load per tile.
"""

import sys

if "/opt/trn_rl_repo" not in sys.path:
    sys.path.insert(0, "/opt/trn_rl_repo")

from contextlib import ExitStack

import numpy as np

import concourse.bass as bass
import concourse.mybir as mybir
import concourse.tile as tile
from concourse import bacc
from concourse.bass_utils import run_bass_kernel_spmd
from concourse.tile_rust import add_dep_helper

N_TOK, D_IN, D_OUT = 8192, 4096, 4096
TOK_WAYS, OUT_WAYS = 4, 2
N_CORES = TOK_WAYS * OUT_WAYS
TOK_SH = N_TOK // TOK_WAYS    # 2048 tokens per core
OUT_SH = D_OUT // OUT_WAYS    # 2048 out features per core

P = 128
KT = D_IN // P                # 32 contraction subtiles
NFREE = 512                   # PSUM free dim per matmul
TB = TOK_SH // P              # 16 token tiles
NB = OUT_SH // NFREE          # 4 feature blocks
RPB = NFREE // P              # 4 weight row-tiles per feature block
TB_P1 = 4                     # token tiles handled in the nb-major phase

F16 = mybir.dt.float16
F32 = mybir.dt.float32


def _build(exact_sign: bool):
    """Build the per-core SPMD program."""
    nc = bacc.Bacc("TRN2", target_bir_lowering=False, debug=False,
                   num_devices=N_CORES)
    x = nc.dram_tensor("x", [TOK_SH, D_IN], F32, kind="ExternalInput")
    w = nc.dram_tensor("w", [OUT_SH, D_IN], F32, kind="ExternalInput")
    b = nc.dram_tensor("b", [1, OUT_SH], F32, kind="ExternalInput")
    out = nc.dram_tensor("out", [TOK_SH, OUT_SH], F32, kind="ExternalOutput")

    with ExitStack() as ctx:
        tc = ctx.enter_context(tile.TileContext(nc))
        dram = ctx.enter_context(tc.tile_pool(name="dram", bufs=TB, space="DRAM"))
        wTp = ctx.enter_context(tc.tile_pool(name="wTp", bufs=NB))
        xTp = ctx.enter_context(tc.tile_pool(name="xTp", bufs=2))
        wf32p = ctx.enter_context(
            tc.tile_pool(name="wf32p", bufs=3 if exact_sign else 4))
        f16tmp = ctx.enter_context(
            tc.tile_pool(name="f16tmp", bufs=5 if exact_sign else 4))
        osbp = ctx.enter_context(tc.tile_pool(name="osbp", bufs=2))
        constp = ctx.enter_context(tc.tile_pool(name="constp", bufs=1))
        mmps = ctx.enter_context(tc.tile_pool(name="mmps", bufs=6, space="PSUM"))

        def sign_half_tile(dst_f16, src_f32, tmp_pool, tag):
            """dst = 0.5*sign(src) (exact) or (src>0)-0.5 (fast, zero-free)."""
            if exact_sign:
                t1 = tmp_pool.tile(list(dst_f16.shape), F16, tag=tag, name=f"{tag}_t1")
                t2 = tmp_pool.tile(list(dst_f16.shape), F16, tag=tag, name=f"{tag}_t2")
                nc.vector.tensor_scalar(t1[:], src_f32, 0.0, None, mybir.AluOpType.is_gt)
                nc.vector.tensor_scalar(t2[:], src_f32, 0.0, None, mybir.AluOpType.is_lt)
                nc.vector.tensor_tensor(t1[:], t1[:], t2[:], mybir.AluOpType.subtract)
                nc.vector.tensor_scalar(dst_f16, t1[:], 0.5, None, mybir.AluOpType.mult)
            else:
                nc.vector.tensor_scalar(
                    dst_f16, src_f32, 0.0, 0.5,
                    mybir.AluOpType.is_gt, mybir.AluOpType.subtract,
                )

        # ---- constants
        ones = constp.tile([1, P], F16)
        nc.gpsimd.memset(ones[:], 1.0)

        # ---- bias row: brow = 0.5*sign(b) as fp16 [1, OUT_SH]
        bf32 = wf32p.tile([1, OUT_SH], F32, tag="wf32", name="bf32")
        nc.scalar.dma_start(bf32[:], b[:])
        brow = constp.tile([1, OUT_SH], F16)
        sign_half_tile(brow[:], bf32[:], constp, "btmp")

        # ---- W prep: half-row tiles [128, D_IN/2] for deep DMA pipelining;
        #      loads on ACT ring, sign on DVE, SBUF->SBUF XBAR transpose on
        #      the SP ring into resident wT tiles.
        HD = D_IN // 2          # 2048 input features per half tile
        HKT = KT // 2           # 16 kt slabs per half tile
        wT = [wTp.tile([P, KT, NFREE], F16, tag="wT", name=f"wT{i}")
              for i in range(NB)]
        for r in range(OUT_SH // P):
            nb, rr = divmod(r, RPB)
            for h in range(2):
                wf32 = wf32p.tile([P, HD], F32, tag="wf32", name="wf32")
                nc.scalar.dma_start(
                    wf32[:], w[r * P : (r + 1) * P, h * HD : (h + 1) * HD])
                wsgn = f16tmp.tile([P, HD], F16, tag="wsgn", name="wsgn")
                sign_half_tile(wsgn[:], wf32[:], f16tmp, "wsgn")
                nc.sync.dma_start_transpose(
                    wT[nb][:, h * HKT : (h + 1) * HKT, rr * P : (rr + 1) * P],
                    wsgn[:])

        # ---- x: SWDGE cast fp32 -> fp16 chunks in DRAM.
        # Chunks 0-1 start immediately; later ones are paced behind xT XBAR
        # loads so the cast flood doesn't starve W prep on the SDMA engines.
        x16 = [None] * TB
        cast_issued = [False] * TB

        def issue_cast(t, dep=None):
            if cast_issued[t]:
                return
            cast_issued[t] = True
            ch = dram.tile([P, D_IN], F16, tag="x16", name=f"x16_{t}")
            inst = nc.gpsimd.dma_start(ch[:], x[t * P : (t + 1) * P, :])
            if dep is not None:
                add_dep_helper(inst.ins, dep.ins, sync=True,
                               reason="pace x cast behind xT consumption")
            x16[t] = ch

        issue_cast(0)
        issue_cast(1)
        issue_cast(2)

        def mm_block(t, nb, xT):
            """One [128-token, 512-feature] output block."""
            psum = mmps.tile([P, NFREE], F32, tag="mm", name="psum")
            nc.tensor.matmul(
                psum[:], ones[:], brow[:, nb * NFREE : (nb + 1) * NFREE],
                start=True, stop=False,
            )
            for kt in range(KT):
                nc.tensor.matmul(
                    psum[:], xT[:, kt, :], wT[nb][:, kt, :],
                    start=False, stop=(kt == KT - 1),
                )
            osb = osbp.tile([P, NFREE], F32, tag="osb", name="osb")
            nc.vector.tensor_scalar(osb[:], psum[:], 2.0, None, mybir.AluOpType.mult)
            # SWDGE for output writes: keeps the two HWDGE rings free for
            # W loads (ACT) and XBAR transposes (SP)
            nc.gpsimd.dma_start(
                out[t * P : (t + 1) * P, nb * NFREE : (nb + 1) * NFREE], osb[:]
            )

        first_load_done = [False] * TB

        def load_xT(t):
            xT = xTp.tile([P, KT, P], F16, tag="xT", name=f"xT_{t}")
            inst = nc.sync.dma_start_transpose(xT[:], x16[t][:])
            if not first_load_done[t]:
                first_load_done[t] = True
                if t + 2 < TB:
                    issue_cast(t + 2, dep=inst)
            return xT

        # Phase 1: first TB_P1 token tiles, feature-block-major, so matmuls
        # start as soon as wT[0] (W rows 0-3) is ready.
        for nb in range(NB):
            for t in range(TB_P1):
                mm_block(t, nb, load_xT(t))

        # Phase 2: remaining token tiles, token-major (one xT load each).
        for t in range(TB_P1, TB):
            xT = load_xT(t)
            for nb in range(NB):
                mm_block(t, nb, xT)

    nc.finalize()
    return nc


_cache = {}


def _get_nc(exact_sign: bool):
    if exact_sign not in _cache:
        _cache[exact_sign] = _build(exact_sign)
    return _cache[exact_sign]


def kernel(x: np.ndarray, weight: np.ndarray, bias: np.ndarray) -> np.ndarray:
    x = np.ascontiguousarray(np.asarray(x, dtype=np.float32))
    weight = np.ascontiguousarray(np.asarray(weight, dtype=np.float32))
    bias = np.ascontiguousarray(np.asarray(bias, dtype=np.float32))
    assert x.shape == (N_TOK, D_IN) and weight.shape == (D_OUT, D_IN)

    # (w > 0) - 0.5 equals 0.5*sign(w) only when no exact zeros exist;
    # fall back to the exact 3-op sign variant otherwise.
    exact_sign = bool((weight == 0.0).any() or (bias == 0.0).any())
    nc = _get_nc(exact_sign)

    in_maps = []
    for tg in range(TOK_WAYS):
        for og in range(OUT_WAYS):
            in_maps.append({
                "x": np.ascontiguousarray(x[tg * TOK_SH : (tg + 1) * TOK_SH, :]),
                "w": np.ascontiguousarray(weight[og * OUT_SH : (og + 1) * OUT_SH, :]),
                "b": np.ascontiguousarray(bias[og * OUT_SH : (og + 1) * OUT_SH].reshape(1, OUT_SH)),
            })

    res = run_bass_kernel_spmd(nc, in_maps, list(range(N_CORES)))

    out = np.empty((N_TOK, D_OUT), dtype=np.float32)
    c = 0
    for tg in range(TOK_WAYS):
        for og in range(OUT_WAYS):
            out[tg * TOK_SH : (tg + 1) * TOK_SH, og * OUT_SH : (og + 1) * OUT_SH] = \
                res.results[c]["out"]
            c += 1
    return out


# revision 11
# speedup vs baseline: 1.1786x; 1.0993x over previous
"""BinLinear TRN2 kernel: out = x @ sign(weight).T + sign(bias).

Full shapes: x [8192, 4096] f32, weight [4096, 4096] f32, bias [4096] f32
-> out [8192, 4096] f32.

Sharding (8 NeuronCores): 2D grid, 4-way over tokens x 2-way over output
features. Each core computes out_c = x_c @ sign(w_c).T + sign(b_c) with
x_c [2048, 4096], w_c [2048, 4096], b_c [2048] -> out_c [2048, 2048].
The host only slices inputs and stitches the 4x2 output grid back together.

Per-core device program (fp16 single-pass matmul, everything on device,
designed around few/large DMA ops -- per-op latency, not bandwidth, was
the measured bottleneck of fine-grained variants):
  - weight: SWDGE DMA casts fp32->fp16 DRAM->DRAM in 256-row slabs, XBAR
    dma-transpose (DRAM->SBUF, one 2MB op per slab) into resident
    wT[i-part, kt, o] tiles, then one DVE op per slab binarizes in place:
    (w16 > 0) - 0.5 = 0.5*sign(w).  (fp16 compare handles subnormals
    exactly -- verified on HW; exact-zero weights are handled by a
    fallback variant selected on the host.)
  - x: SWDGE DMA cast fp32->fp16 into 256-token DRAM chunks (first two
    immediately, the rest paced behind xT consumption so the cast flood
    never starves weight prep), then one XBAR transpose per chunk into
    xT[i-part, kt, tok] SBUF tiles.
  - PE: per [128-token, 512-feature] PSUM tile: one K=1 matmul seeds the
    bias row (ones^T @ 0.5*sign(b)), then 32 K=128 fp16 matmuls accumulate
    x @ w_sign.  Copy-back scales by 2 on DVE into fp32 output tiles.
  - Schedule: the first two x-chunks run feature-block-major so matmuls
    start as soon as the first quarter of wT is ready; remaining chunks
    run token-major with kt-outer/feature-inner matmul ordering.
"""

import sys

if "/opt/trn_rl_repo" not in sys.path:
    sys.path.insert(0, "/opt/trn_rl_repo")

from contextlib import ExitStack

import numpy as np

import concourse.bass as bass
import concourse.mybir as mybir
import concourse.tile as tile
from concourse import bacc
from concourse.bass_utils import run_bass_kernel_spmd
from concourse.tile_rust import add_dep_helper

N_TOK, D_IN, D_OUT = 8192, 4096, 4096
TOK_WAYS, OUT_WAYS = 4, 2
N_CORES = TOK_WAYS * OUT_WAYS
TOK_SH = N_TOK // TOK_WAYS    # 2048 tokens per core
OUT_SH = D_OUT // OUT_WAYS    # 2048 out features per core

P = 128
KT = D_IN // P                # 32 contraction subtiles
NFREE = 512                   # PSUM free dim per matmul
NB = OUT_SH // NFREE          # 4 feature blocks
SC = 256                      # tokens per x super-chunk
NSC = TOK_SH // SC            # 8 super-chunks
SC_P1 = 2                     # super-chunks handled in the nb-major phase
WSLAB = 256                   # weight rows per cast/transpose slab
NWS = OUT_SH // WSLAB         # 8 weight slabs

F16 = mybir.dt.float16
F32 = mybir.dt.float32


def _build(exact_sign: bool):
    """Build the per-core SPMD program."""
    nc = bacc.Bacc("TRN2", target_bir_lowering=False, debug=False,
                   num_devices=N_CORES)
    x = nc.dram_tensor("x", [TOK_SH, D_IN], F32, kind="ExternalInput")
    w = nc.dram_tensor("w", [OUT_SH, D_IN], F32, kind="ExternalInput")
    b = nc.dram_tensor("b", [1, OUT_SH], F32, kind="ExternalInput")
    out = nc.dram_tensor("out", [TOK_SH, OUT_SH], F32, kind="ExternalOutput")

    with ExitStack() as ctx:
        tc = ctx.enter_context(tile.TileContext(nc))
        dram = ctx.enter_context(tc.tile_pool(name="dram", bufs=1, space="DRAM"))
        wTp = ctx.enter_context(tc.tile_pool(name="wTp", bufs=NB))
        xTp = ctx.enter_context(tc.tile_pool(name="xTp", bufs=2))
        sgtmp = ctx.enter_context(tc.tile_pool(name="sgtmp", bufs=2))
        osbp = ctx.enter_context(tc.tile_pool(name="osbp", bufs=2))
        constp = ctx.enter_context(tc.tile_pool(name="constp", bufs=1))
        mmps = ctx.enter_context(tc.tile_pool(name="mmps", bufs=8, space="PSUM"))

        def sign_inplace(ap, tmp_shape, tag):
            """ap = 0.5*sign(ap) elementwise on fp16 data (in place)."""
            if exact_sign:
                # (gt - lt)*0.5 handles exact +-0 -> 0
                t1 = sgtmp.tile(tmp_shape, F16, tag=tag, name=f"{tag}_t")
                nc.vector.tensor_scalar(t1[:], ap, 0.0, None, mybir.AluOpType.is_lt)
                nc.vector.tensor_scalar(ap, ap, 0.0, None, mybir.AluOpType.is_gt)
                nc.vector.tensor_tensor(ap, ap, t1[:], mybir.AluOpType.subtract)
                nc.vector.tensor_scalar(ap, ap, 0.5, None, mybir.AluOpType.mult)
            else:
                nc.vector.tensor_scalar(
                    ap, ap, 0.0, 0.5,
                    mybir.AluOpType.is_gt, mybir.AluOpType.subtract,
                )

        # ---- constants
        ones = constp.tile([1, P], F16)
        nc.gpsimd.memset(ones[:], 1.0)

        # ---- bias row: brow = 0.5*sign(b) as fp16 [1, OUT_SH]
        bf32 = constp.tile([1, OUT_SH], F32)
        nc.scalar.dma_start(bf32[:], b[:])
        brow = constp.tile([1, OUT_SH], F16)
        if exact_sign:
            bt = constp.tile([1, OUT_SH], F16)
            nc.vector.tensor_scalar(bt[:], bf32[:], 0.0, None, mybir.AluOpType.is_lt)
            nc.vector.tensor_scalar(brow[:], bf32[:], 0.0, None, mybir.AluOpType.is_gt)
            nc.vector.tensor_tensor(brow[:], brow[:], bt[:], mybir.AluOpType.subtract)
            nc.vector.tensor_scalar(brow[:], brow[:], 0.5, None, mybir.AluOpType.mult)
        else:
            nc.vector.tensor_scalar(
                brow[:], bf32[:], 0.0, 0.5,
                mybir.AluOpType.is_gt, mybir.AluOpType.subtract,
            )

        # ---- SWDGE cast ops (DRAM->DRAM fp32->fp16), explicitly ordered:
        # weight slabs for wT[0] first, then the first two x chunks, then
        # the remaining weight slabs; later x chunks are paced by deps.
        last_swdge = [None]

        def swdge_cast(dst_tile, src_ap, pace_dep=None):
            inst = nc.gpsimd.dma_start(dst_tile[:], src_ap)
            if last_swdge[0] is not None:
                add_dep_helper(inst.ins, last_swdge[0].ins, sync=False,
                               reason="SWDGE cast order")
            if pace_dep is not None:
                add_dep_helper(inst.ins, pace_dep.ins, sync=True,
                               reason="pace x cast behind xT consumption")
            last_swdge[0] = inst
            return inst

        w16 = [dram.tile([WSLAB, D_IN], F16, tag="w16", name=f"w16_{j}", bufs=NWS)
               for j in range(NWS)]
        x16 = [None] * NSC
        cast_issued = [False] * NSC

        def issue_x_cast(s, dep=None):
            if cast_issued[s]:
                return
            cast_issued[s] = True
            ch = dram.tile([SC, D_IN], F16, tag="x16", name=f"x16_{s}", bufs=NSC)
            swdge_cast(ch, x[s * SC : (s + 1) * SC, :], pace_dep=dep)
            x16[s] = ch

        # weight slabs 0,1 (wT[0]) -> x chunks 0,1 -> weight slabs 2..7
        for j in (0, 1):
            swdge_cast(w16[j], w[j * WSLAB : (j + 1) * WSLAB, :])
        issue_x_cast(0)
        issue_x_cast(1)
        for j in range(2, NWS):
            swdge_cast(w16[j], w[j * WSLAB : (j + 1) * WSLAB, :])

        # ---- wT: XBAR transpose each weight slab DRAM->SBUF, then one DVE
        # op binarizes the slab in place.
        wT = [wTp.tile([P, KT, NFREE], F16, tag="wT", name=f"wT{i}")
              for i in range(NB)]
        for j in range(NWS):
            nb, jj = j // 2, j % 2
            dst = wT[nb][:, :, jj * WSLAB : (jj + 1) * WSLAB]
            nc.sync.dma_start_transpose(dst, w16[j][:])
            sign_inplace(dst, [P, KT, WSLAB], "wsg")

        first_load_done = [False] * NSC

        def load_xT(s):
            xT = xTp.tile([P, KT, SC], F16, tag="xT", name=f"xT_{s}")
            inst = nc.sync.dma_start_transpose(xT[:], x16[s][:])
            if not first_load_done[s]:
                first_load_done[s] = True
                if s + 2 < NSC:
                    issue_x_cast(s + 2, dep=inst)
            return xT

        def bias_mm(psum, nb):
            nc.tensor.matmul(
                psum[:], ones[:], brow[:, nb * NFREE : (nb + 1) * NFREE],
                start=True, stop=False,
            )

        def copy_out_small(psum, s, half, nb):
            osb = osbp.tile([P, NFREE], F32, tag="osb_s", name="osb_s")
            nc.vector.tensor_scalar(osb[:], psum[:], 2.0, None, mybir.AluOpType.mult)
            r0 = s * SC + half * P
            nc.scalar.dma_start(
                out[r0 : r0 + P, nb * NFREE : (nb + 1) * NFREE], osb[:])

        # ---- Phase 1: super-chunks 0..SC_P1-1 stay resident in SBUF;
        # iterate feature-block-major so matmuls start on wT[0].
        xTs_p1 = [load_xT(s) for s in range(SC_P1)]
        for nb in range(NB):
            for s in range(SC_P1):
                for half in range(2):
                    psum = mmps.tile([P, NFREE], F32, tag="mm", name="psum")
                    bias_mm(psum, nb)
                    lhsT = xTs_p1[s]
                    for kt in range(KT):
                        nc.tensor.matmul(
                            psum[:],
                            lhsT[:, kt, half * P : (half + 1) * P],
                            wT[nb][:, kt, :],
                            start=False, stop=(kt == KT - 1),
                        )
                    copy_out_small(psum, s, half, nb)

        # ---- Phase 2: remaining super-chunks, token-major; kt-outer /
        # feature-inner so one stationary load feeds four matmuls, grouped
        # 1MB output writes.
        for s in range(SC_P1, NSC):
            xT = load_xT(s)
            for half in range(2):
                psums = [mmps.tile([P, NFREE], F32, tag="mm", name=f"psum{i}")
                         for i in range(NB)]
                for nb in range(NB):
                    bias_mm(psums[nb], nb)
                for kt in range(KT):
                    lhsT = xT[:, kt, half * P : (half + 1) * P]
                    for nb in range(NB):
                        nc.tensor.matmul(
                            psums[nb][:], lhsT, wT[nb][:, kt, :],
                            start=False, stop=(kt == KT - 1),
                        )
                osb = osbp.tile([P, OUT_SH], F32, tag="osb_b", name="osb_b")
                for nb in range(NB):
                    nc.vector.tensor_scalar(
                        osb[:, nb * NFREE : (nb + 1) * NFREE], psums[nb][:],
                        2.0, None, mybir.AluOpType.mult)
                r0 = s * SC + half * P
                nc.scalar.dma_start(out[r0 : r0 + P, :], osb[:])

    nc.finalize()
    return nc


_cache = {}


def _get_nc(exact_sign: bool):
    if exact_sign not in _cache:
        _cache[exact_sign] = _build(exact_sign)
    return _cache[exact_sign]


def kernel(x: np.ndarray, weight: np.ndarray, bias: np.ndarray) -> np.ndarray:
    x = np.ascontiguousarray(np.asarray(x, dtype=np.float32))
    weight = np.ascontiguousarray(np.asarray(weight, dtype=np.float32))
    bias = np.ascontiguousarray(np.asarray(bias, dtype=np.float32))
    assert x.shape == (N_TOK, D_IN) and weight.shape == (D_OUT, D_IN)

    # (w > 0) - 0.5 equals 0.5*sign(w) only when no exact zeros exist;
    # fall back to the exact 3-op sign variant otherwise.
    exact_sign = bool((weight == 0.0).any() or (bias == 0.0).any())
    nc = _get_nc(exact_sign)

    in_maps = []
    for tg in range(TOK_WAYS):
        for og in range(OUT_WAYS):
            in_maps.append({
                "x": np.ascontiguousarray(x[tg * TOK_SH : (tg + 1) * TOK_SH, :]),
                "w": np.ascontiguousarray(weight[og * OUT_SH : (og + 1) * OUT_SH, :]),
                "b": np.ascontiguousarray(bias[og * OUT_SH : (og + 1) * OUT_SH].reshape(1, OUT_SH)),
            })

    res = run_bass_kernel_spmd(nc, in_maps, list(range(N_CORES)))

    out = np.empty((N_TOK, D_OUT), dtype=np.float32)
    c = 0
    for tg in range(TOK_WAYS):
        for og in range(OUT_WAYS):
            out[tg * TOK_SH : (tg + 1) * TOK_SH, og * OUT_SH : (og + 1) * OUT_SH] = \
                res.results[c]["out"]
            c += 1
    return out


# revision 17
# speedup vs baseline: 1.2543x; 1.0643x over previous
"""BinLinear TRN2 kernel: out = x @ sign(weight).T + sign(bias).

Full shapes: x [8192, 4096] f32, weight [4096, 4096] f32, bias [4096] f32
-> out [8192, 4096] f32.

Sharding (8 NeuronCores): 2D grid, 4-way over tokens x 2-way over output
features. Each core computes out_c = x_c @ sign(w_c).T + sign(b_c) with
x_c [2048, 4096], w_c [2048, 4096], b_c [2048] -> out_c [2048, 2048].
The host only slices inputs and stitches the 4x2 output grid back together.

Per-core device program (fp16 single-pass matmul, everything on device,
designed around few/large DMA ops -- per-op latency, not bandwidth, was
the measured bottleneck of fine-grained variants):
  - weight: SWDGE DMA casts fp32->fp16 DRAM->DRAM in 256-row slabs, XBAR
    dma-transpose (DRAM->SBUF, one 2MB op per slab) into resident
    wT[i-part, kt, o] tiles, then one DVE op per slab binarizes in place:
    (w16 > 0) - 0.5 = 0.5*sign(w).  (fp16 compare handles subnormals
    exactly -- verified on HW; exact-zero weights are handled by a
    fallback variant selected on the host.)
  - x: SWDGE DMA cast fp32->fp16 into 256-token DRAM chunks (first two
    immediately, the rest paced behind xT consumption so the cast flood
    never starves weight prep), then one XBAR transpose per chunk into
    xT[i-part, kt, tok] SBUF tiles.
  - PE: per [128-token, 512-feature] PSUM tile: one K=1 matmul seeds the
    bias row (ones^T @ 0.5*sign(b)), then 32 K=128 fp16 matmuls accumulate
    x @ w_sign.  Copy-back scales by 2 on DVE into fp32 output tiles.
  - Schedule: the first two x-chunks run feature-block-major so matmuls
    start as soon as the first quarter of wT is ready; remaining chunks
    run token-major with kt-outer/feature-inner matmul ordering.
"""

import sys

if "/opt/trn_rl_repo" not in sys.path:
    sys.path.insert(0, "/opt/trn_rl_repo")

from contextlib import ExitStack

import numpy as np

import concourse.bass as bass
import concourse.mybir as mybir
import concourse.tile as tile
from concourse import bacc
from concourse.bass_utils import run_bass_kernel_spmd
from concourse.tile_rust import add_dep_helper

N_TOK, D_IN, D_OUT = 8192, 4096, 4096
TOK_WAYS, OUT_WAYS = 4, 2
N_CORES = TOK_WAYS * OUT_WAYS
TOK_SH = N_TOK // TOK_WAYS    # 2048 tokens per core
OUT_SH = D_OUT // OUT_WAYS    # 2048 out features per core

P = 128
KT = D_IN // P                # 32 contraction subtiles
NFREE = 512                   # PSUM free dim per matmul
NB = OUT_SH // NFREE          # 4 feature blocks
SC = 256                      # tokens per x super-chunk
NSC = TOK_SH // SC            # 8 super-chunks
SC_P1 = 2                     # super-chunks handled in the nb-major phase
WSLAB = 256                   # weight rows per cast/transpose slab
NWS = OUT_SH // WSLAB         # 8 weight slabs

F16 = mybir.dt.float16
F32 = mybir.dt.float32


def _build(exact_sign: bool):
    """Build the per-core SPMD program."""
    nc = bacc.Bacc("TRN2", target_bir_lowering=False, debug=False,
                   num_devices=N_CORES)
    x = nc.dram_tensor("x", [TOK_SH, D_IN], F32, kind="ExternalInput")
    w = nc.dram_tensor("w", [OUT_SH, D_IN], F32, kind="ExternalInput")
    b = nc.dram_tensor("b", [1, OUT_SH], F32, kind="ExternalInput")
    out = nc.dram_tensor("out", [TOK_SH, OUT_SH], F32, kind="ExternalOutput")

    with ExitStack() as ctx:
        tc = ctx.enter_context(tile.TileContext(nc))
        dram = ctx.enter_context(tc.tile_pool(name="dram", bufs=1, space="DRAM"))
        wTp = ctx.enter_context(tc.tile_pool(name="wTp", bufs=NB))
        xTp = ctx.enter_context(tc.tile_pool(name="xTp", bufs=2))
        w16p = ctx.enter_context(tc.tile_pool(name="w16p", bufs=2))
        sgtmp = ctx.enter_context(tc.tile_pool(name="sgtmp", bufs=2))
        osbp = ctx.enter_context(tc.tile_pool(name="osbp", bufs=2))
        constp = ctx.enter_context(tc.tile_pool(name="constp", bufs=1))
        mmps = ctx.enter_context(tc.tile_pool(name="mmps", bufs=8, space="PSUM"))

        def sign_inplace(ap, tmp_shape, tag):
            """ap = 0.5*sign(ap) elementwise on fp16 data (in place)."""
            if exact_sign:
                # (gt - lt)*0.5 handles exact +-0 -> 0
                t1 = sgtmp.tile(tmp_shape, F16, tag=tag, name=f"{tag}_t")
                nc.vector.tensor_scalar(t1[:], ap, 0.0, None, mybir.AluOpType.is_lt)
                nc.vector.tensor_scalar(ap, ap, 0.0, None, mybir.AluOpType.is_gt)
                nc.vector.tensor_tensor(ap, ap, t1[:], mybir.AluOpType.subtract)
                nc.vector.tensor_scalar(ap, ap, 0.5, None, mybir.AluOpType.mult)
            else:
                nc.vector.tensor_scalar(
                    ap, ap, 0.0, 0.5,
                    mybir.AluOpType.is_gt, mybir.AluOpType.subtract,
                )

        # ---- constants
        ones = constp.tile([1, P], F16)
        nc.gpsimd.memset(ones[:], 1.0)

        # ---- bias row: brow = 0.5*sign(b) as fp16 [1, OUT_SH]
        # (bf32 is transient; it shares the w16 staging slots)
        bf32 = w16p.tile([1, OUT_SH], F32, tag="w16", name="bf32")
        nc.scalar.dma_start(bf32[:], b[:])
        brow = constp.tile([1, OUT_SH], F16)
        if exact_sign:
            bt = constp.tile([1, OUT_SH], F16)
            nc.vector.tensor_scalar(bt[:], bf32[:], 0.0, None, mybir.AluOpType.is_lt)
            nc.vector.tensor_scalar(brow[:], bf32[:], 0.0, None, mybir.AluOpType.is_gt)
            nc.vector.tensor_tensor(brow[:], brow[:], bt[:], mybir.AluOpType.subtract)
            nc.vector.tensor_scalar(brow[:], brow[:], 0.5, None, mybir.AluOpType.mult)
        else:
            nc.vector.tensor_scalar(
                brow[:], bf32[:], 0.0, 0.5,
                mybir.AluOpType.is_gt, mybir.AluOpType.subtract,
            )

        # ---- SWDGE cast ops (DRAM->DRAM fp32->fp16), explicitly ordered:
        # weight slabs for wT[0] first, then the first two x chunks, then
        # the remaining weight slabs; later x chunks are paced by deps.
        last_swdge = [None]

        def swdge_cast(dst_tile, src_ap, pace_dep=None):
            inst = nc.gpsimd.dma_start(dst_tile[:], src_ap)
            if last_swdge[0] is not None:
                add_dep_helper(inst.ins, last_swdge[0].ins, sync=False,
                               reason="SWDGE cast order")
            if pace_dep is not None:
                add_dep_helper(inst.ins, pace_dep.ins, sync=True,
                               reason="pace x cast behind xT consumption")
            last_swdge[0] = inst
            return inst

        x16 = [None] * NSC
        cast_issued = [False] * NSC

        def issue_x_cast(s, dep=None):
            if cast_issued[s]:
                return
            cast_issued[s] = True
            ch = dram.tile([SC, D_IN], F16, tag="x16", name=f"x16_{s}", bufs=NSC)
            swdge_cast(ch, x[s * SC : (s + 1) * SC, :], pace_dep=dep)
            x16[s] = ch

        # ---- wT: SWDGE-cast each 128-row weight slab fp32 DRAM -> fp16
        # SBUF, XBAR SBUF->SBUF transpose into wT, one DVE op binarizes the
        # slab in place.  Total HBM traffic for W: one fp32 read.
        wT = [wTp.tile([P, KT, NFREE], F16, tag="wT", name=f"wT{i}")
              for i in range(NB)]
        NSLAB = OUT_SH // P      # 16 weight slabs of 128 rows

        def w_slab(j):
            w16 = w16p.tile([P, D_IN], F16, tag="w16", name=f"w16_{j}")
            swdge_cast(w16, w[j * P : (j + 1) * P, :])
            nb, jj = j // NB, j % NB
            dst = wT[nb][:, :, jj * P : (jj + 1) * P]
            nc.sync.dma_start_transpose(dst, w16[:])
            sign_inplace(dst, [P, KT, P], "wsg")

        first_load_done = [False] * NSC

        def load_xT(s, pace=True):
            xT = xTp.tile([P, KT, SC], F16, tag="xT", name=f"xT_{s}")
            inst = nc.sync.dma_start_transpose(xT[:], x16[s][:])
            if not first_load_done[s]:
                first_load_done[s] = True
                if pace and s + 2 < NSC:
                    issue_x_cast(s + 2, dep=inst)
            return xT, inst

        # SWDGE cast order: weight slabs 0..3 (wT[0]), x chunks 0,1, the
        # remaining weight slabs, then paced x chunks.  SP-ring order:
        # wT XBARs 0..3, xT XBARs 0,1, wT XBARs 4..15 -- so phase 1 can
        # start while the rest of W prep streams in behind it.
        for j in range(4):
            w_slab(j)
        issue_x_cast(0)
        issue_x_cast(1)
        xT0, xT0_inst = load_xT(0, pace=False)
        xT1, xT1_inst = load_xT(1, pace=False)
        for j in range(4, NSLAB):
            w_slab(j)
        issue_x_cast(2, dep=xT0_inst)
        issue_x_cast(3, dep=xT1_inst)

        def bias_mm(psum, nb):
            nc.tensor.matmul(
                psum[:], ones[:], brow[:, nb * NFREE : (nb + 1) * NFREE],
                start=True, stop=False,
            )

        def copy_out_small(psum, s, half, nb):
            osb = osbp.tile([P, NFREE], F32, tag="osb_s", name="osb_s")
            nc.vector.tensor_scalar(osb[:], psum[:], 2.0, None, mybir.AluOpType.mult)
            r0 = s * SC + half * P
            nc.scalar.dma_start(
                out[r0 : r0 + P, nb * NFREE : (nb + 1) * NFREE], osb[:])

        # ---- Phase 1: super-chunks 0..SC_P1-1 stay resident in SBUF;
        # iterate feature-block-major so matmuls start on wT[0].
        xTs_p1 = [xT0, xT1]
        for nb in range(NB):
            for s in range(SC_P1):
                for half in range(2):
                    psum = mmps.tile([P, NFREE], F32, tag="mm", name="psum")
                    bias_mm(psum, nb)
                    lhsT = xTs_p1[s]
                    for kt in range(KT):
                        nc.tensor.matmul(
                            psum[:],
                            lhsT[:, kt, half * P : (half + 1) * P],
                            wT[nb][:, kt, :],
                            start=False, stop=(kt == KT - 1),
                        )
                    copy_out_small(psum, s, half, nb)

        # ---- Phase 2: remaining super-chunks, token-major; kt-outer /
        # feature-inner so one stationary load feeds four matmuls, grouped
        # 1MB output writes.
        for s in range(SC_P1, NSC):
            xT, _ = load_xT(s)
            for half in range(2):
                psums = [mmps.tile([P, NFREE], F32, tag="mm", name=f"psum{i}")
                         for i in range(NB)]
                for nb in range(NB):
                    bias_mm(psums[nb], nb)
                for kt in range(KT):
                    lhsT = xT[:, kt, half * P : (half + 1) * P]
                    for nb in range(NB):
                        nc.tensor.matmul(
                            psums[nb][:], lhsT, wT[nb][:, kt, :],
                            start=False, stop=(kt == KT - 1),
                        )
                osb = osbp.tile([P, OUT_SH], F32, tag="osb_b", name="osb_b")
                for nb in range(NB):
                    nc.vector.tensor_scalar(
                        osb[:, nb * NFREE : (nb + 1) * NFREE], psums[nb][:],
                        2.0, None, mybir.AluOpType.mult)
                r0 = s * SC + half * P
                nc.scalar.dma_start(out[r0 : r0 + P, :], osb[:])

    nc.finalize()
    return nc


_cache = {}


def _get_nc(exact_sign: bool):
    if exact_sign not in _cache:
        _cache[exact_sign] = _build(exact_sign)
    return _cache[exact_sign]


def kernel(x: np.ndarray, weight: np.ndarray, bias: np.ndarray) -> np.ndarray:
    x = np.ascontiguousarray(np.asarray(x, dtype=np.float32))
    weight = np.ascontiguousarray(np.asarray(weight, dtype=np.float32))
    bias = np.ascontiguousarray(np.asarray(bias, dtype=np.float32))
    assert x.shape == (N_TOK, D_IN) and weight.shape == (D_OUT, D_IN)

    # (w > 0) - 0.5 equals 0.5*sign(w) only when no exact zeros exist;
    # fall back to the exact 3-op sign variant otherwise.
    exact_sign = bool((weight == 0.0).any() or (bias == 0.0).any())
    nc = _get_nc(exact_sign)

    in_maps = []
    for tg in range(TOK_WAYS):
        for og in range(OUT_WAYS):
            in_maps.append({
                "x": np.ascontiguousarray(x[tg * TOK_SH : (tg + 1) * TOK_SH, :]),
                "w": np.ascontiguousarray(weight[og * OUT_SH : (og + 1) * OUT_SH, :]),
                "b": np.ascontiguousarray(bias[og * OUT_SH : (og + 1) * OUT_SH].reshape(1, OUT_SH)),
            })

    res = run_bass_kernel_spmd(nc, in_maps, list(range(N_CORES)))

    out = np.empty((N_TOK, D_OUT), dtype=np.float32)
    c = 0
    for tg in range(TOK_WAYS):
        for og in range(OUT_WAYS):
            out[tg * TOK_SH : (tg + 1) * TOK_SH, og * OUT_SH : (og + 1) * OUT_SH] = \
                res.results[c]["out"]
            c += 1
    return out
